# revision 30
# baseline (speedup 1.0000x reference)
"""Trainium2 Bass kernel for RelPatchAttention2D (THW).

Problem: q,k,v (4,16,16,128,128) f32. Patchify into 4096 patches/batch of
dim 1024. sim[q,k] = (qk+s)/(qq+kk-qk+s); tqk[k] = mean_q sim; out = tqk * v.

Sharding (no collectives): 8 cores = 4 batches x 2 key-halves. Each core:
full queries (4096) x its 2048 keys. Host prepares transposed bf16 patch
matrices (with two augmentation rows), gathers/unpatchifies outputs.

Per-core kernel (layout: keys on partitions, queries on free dim).
Loop nest: kt PAIRS outer, qt inner — each pair's mean over queries
completes mid-kernel, so output work spreads instead of piling into a
tail. Q^T stays fully resident in SBUF; V streams per pair.

  per (qt,kt) tile [128 keys x 512 queries]:
    PE:  8 bf16 matmuls (d-chunks; stationary -K^T, moving Q^T)
         accumulate P = -qk in PSUM
    ACT: N = -P + s   (PSUM->SBUF numerator read, overlapped)
  per group of 4 tiles (issued one tile late to hide the ACT read):
    PE:  4 aug matmuls (K=2 rows: qq_q*1 + 1*kk_k) onto the 4 banks,
         row-tiled via tile_position=(32i,0) -> D = qq+kk-qk
    DVE: r = reciprocal_approx_fast(D)
         acc[:,qt] = sum_q N*r   (scalar_tensor_tensor with accum)
  per kt at its last qt: tqk = rowsum(acc)/4096; out = tqk*v (ACT scale)

Numerics: N comes from the PSUM qk itself, so qq/kk quantization (bf16)
only perturbs the denominator - a benign RELATIVE error on sim. The N*r
form keeps the reciprocal's error relative to sim as well (no catastrophic
cancellation in sum(A/D)-4096).
"""
import os
import sys

import numpy as np

sys.path.insert(0, '/opt/trn_rl_repo')

SMOOTH = 1e-05
B, T, C, H, W = 4, 16, 16, 128, 128
SH = SW = 16
PH = PW = 8
NPATCH = T * SH * SW        # 4096 patches per batch (queries)
DPATCH = C * PH * PW        # 1024
KEYS_PER_CORE = NPATCH // 2  # 2048
N_CORES = 8

QT_TILES = NPATCH // 512     # 8
KT_TILES = KEYS_PER_CORE // 128  # 16
DC = DPATCH // 128           # 8 contraction chunks
GRP = 4                      # tiles per aug group (row-tiled aug packing)


# ----------------------------------------------------------------- host side

def _patchify_mat(x):
    # (B,T,C,H,W) -> (B, 4096, 1024), patch index = ((t*16+sh)*16+sw)
    xp = x.reshape(B, T, C, SH, PH, SW, PW).transpose(0, 1, 3, 5, 2, 4, 6)
    return np.ascontiguousarray(xp).reshape(B, NPATCH, DPATCH)


def _unpatchify_mat(p):
    # (B, 4096, 1024) -> (B,T,C,H,W)
    x = p.reshape(B, T, SH, SW, C, PH, PW).transpose(0, 1, 4, 2, 5, 3, 6)
    return np.ascontiguousarray(x).reshape(B, T, C, H, W)


def _host_prepare(q, k, v):
    import ml_dtypes
    QP = _patchify_mat(q)
    KP = _patchify_mat(k)
    VP = _patchify_mat(v)
    qq = np.square(QP, dtype=np.float64).sum(-1).astype(np.float32)
    kk = np.square(KP, dtype=np.float64).sum(-1).astype(np.float32)

    in_maps = []
    for b in range(B):
        qta = np.concatenate(
            [QP[b].T,
             qq[b][None, :],
             np.ones((1, NPATCH), np.float32)], axis=0)
        qta = np.ascontiguousarray(qta).astype(ml_dtypes.bfloat16)
        for half in range(2):
            sl = slice(half * KEYS_PER_CORE, (half + 1) * KEYS_PER_CORE)
            kta = np.concatenate(
                [-KP[b, sl].T,
                 np.ones((1, KEYS_PER_CORE), np.float32),
                 kk[b, sl][None, :]], axis=0)
            kta = np.ascontiguousarray(kta).astype(ml_dtypes.bfloat16)
            in_maps.append({
                'qta': qta,
                'kta': kta,
                'vp': np.ascontiguousarray(VP[b, sl]),
            })
    return in_maps


def _host_finish(outs):
    full = np.empty((B, NPATCH, DPATCH), np.float32)
    for b in range(B):
        full[b, :KEYS_PER_CORE] = outs[2 * b]
        full[b, KEYS_PER_CORE:] = outs[2 * b + 1]
    return _unpatchify_mat(full)


# --------------------------------------------------------------- bass kernel

def build_nc():
    import concourse.bass as bass  # noqa: F401
    import concourse.mybir as mybir
    import concourse.tile as tile
    from concourse import bacc

    f32 = mybir.dt.float32
    bf16 = mybir.dt.bfloat16
    Alu = mybir.AluOpType
    Act = mybir.ActivationFunctionType

    nc = bacc.Bacc(
        "TRN2",
        target_bir_lowering=False,
        debug=False,
        enable_asserts=False,
        num_devices=N_CORES,
    )

    qta = nc.dram_tensor("qta", [DPATCH + 2, NPATCH], bf16, kind="ExternalInput").ap()
    kta = nc.dram_tensor("kta", [DPATCH + 2, KEYS_PER_CORE], bf16, kind="ExternalInput").ap()
    vp = nc.dram_tensor("vp", [KEYS_PER_CORE, DPATCH], f32, kind="ExternalInput").ap()
    out = nc.dram_tensor("out", [KEYS_PER_CORE, DPATCH], f32, kind="ExternalOutput").ap()

    with tile.TileContext(nc) as tc:
        with (
            tc.tile_pool(name="ktp", bufs=1) as ktp,
            tc.tile_pool(name="qp", bufs=1) as qp,
            tc.tile_pool(name="psp", bufs=8, space="PSUM") as psp,
            tc.tile_pool(name="np_", bufs=6) as np_p,
            tc.tile_pool(name="rp", bufs=5) as rp,
            tc.tile_pool(name="scrp", bufs=3) as scrp,
            tc.tile_pool(name="accp", bufs=1) as accp,
            tc.tile_pool(name="wp", bufs=2) as wp,
            tc.tile_pool(name="vvp", bufs=4) as vvp,
            tc.tile_pool(name="outp", bufs=2) as outp,
        ):
            # all Q^T tiles resident; issued in qt order so the first pair's
            # stream stays ahead of the PE
            q_tiles = {}   # (qt, c) -> tile
            q_augs = {}
            for qt in range(QT_TILES):
                qs = slice(qt * 512, (qt + 1) * 512)
                for c in range(DC):
                    t = qp.tile([128, 512], bf16,
                                name=f"qtt{c}_{qt}", tag=f"qtt{c}_{qt}")
                    nc.sync.dma_start(t[:], qta[c * 128:(c + 1) * 128, qs])
                    q_tiles[(qt, c)] = t
                # aug rows replicated at partition offsets 0/32/64/96
                qa = qp.tile([98, 512], bf16, name=f"qaug_{qt}", tag=f"qaug_{qt}")
                for i in range(GRP):
                    nc.sync.dma_start(
                        qa[32 * i:32 * i + 2, :], qta[DPATCH:DPATCH + 2, qs])
                q_augs[qt] = qa

            # resident -K^T chunks + aug rows; first 128 columns first (all
            # tile 0 needs), via the idle GpSimd DMA queue
            kt_tiles = []
            for c in range(DC):
                t = ktp.tile([128, KEYS_PER_CORE], bf16, name=f"ktt{c}", tag=f"ktt{c}")
                nc.gpsimd.dma_start(t[:, 0:128], kta[c * 128:(c + 1) * 128, 0:128])
                kt_tiles.append(t)
            kt_aug = ktp.tile([98, KEYS_PER_CORE], bf16, name="ktaug", tag="ktaug")
            for i in range(GRP):
                nc.gpsimd.dma_start(
                    kt_aug[32 * i:32 * i + 2, :], kta[DPATCH:DPATCH + 2, :])
            for c in range(DC):
                nc.gpsimd.dma_start(
                    kt_tiles[c][:, 128:], kta[c * 128:(c + 1) * 128, 128:])

            # per-kt accumulators: one column per qt, reduced at the end
            acc_tiles = []
            for kt in range(KT_TILES):
                t = accp.tile([128, QT_TILES], f32, name=f"acc{kt}", tag=f"acc{kt}")
                acc_tiles.append(t)

            v_tiles = {}

            def finish_kt(kt):
                red_t = wp.tile([128, 1], f32, name=f"red_{kt}", tag="red")
                nc.vector.tensor_reduce(
                    red_t[:], acc_tiles[kt][:],
                    op=Alu.add, axis=mybir.AxisListType.X)
                w_t = wp.tile([128, 1], f32, name=f"w_{kt}", tag="w")
                nc.scalar.activation(
                    w_t[:], red_t[:], Act.Copy, scale=1.0 / NPATCH)
                o_t = outp.tile([128, DPATCH], f32, name=f"o_{kt}", tag="o")
                nc.scalar.activation(o_t[:], v_tiles[kt][:], Act.Copy, scale=w_t[:])
                nc.sync.dma_start(out[kt * 128:(kt + 1) * 128, :], o_t[:])

            def finish_group(grp):
                """aug matmuls (row-tiled) + recip + STT accum for a group."""
                for i, (ps, n_t, qt, kt) in enumerate(grp):
                    ks = slice(kt * 128, (kt + 1) * 128)
                    nc.tensor.matmul(
                        ps[:],
                        kt_aug[32 * i:32 * i + 2, ks],
                        q_augs[qt][32 * i:32 * i + 2, :],
                        start=False, stop=True,
                        skip_group_check=True,
                        tile_position=(32 * i, 0),
                    )
                for (ps, n_t, qt, kt) in grp:
                    r_t = rp.tile([128, 512], f32, name=f"r_{qt}_{kt}", tag="r")
                    nc.vector.reciprocal_approx_fast(r_t[:], ps[:])
                    scr = scrp.tile([128, 512], f32, name=f"scr_{qt}_{kt}", tag="scr")
                    nc.vector.scalar_tensor_tensor(
                        scr[:], n_t[:], 1.0, r_t[:],
                        op0=Alu.bypass, op1=Alu.mult,
                        accum_out=acc_tiles[kt][:, qt:qt + 1],
                    )
                    if qt == QT_TILES - 1:
                        finish_kt(kt)

            pending = []
            flushed = None
            n_tiles = 0
            for pair in range(KT_TILES // 2):
                kts = (2 * pair, 2 * pair + 1)
                for kt in kts:
                    t = vvp.tile([128, DPATCH], f32, name=f"v_{kt}", tag="v")
                    nc.gpsimd.dma_start(t[:], vp[kt * 128:(kt + 1) * 128, :])
                    v_tiles[kt] = t
                for qt in range(QT_TILES):
                    for kt in kts:
                        ks = slice(kt * 128, (kt + 1) * 128)
                        ps = psp.tile([128, 512], f32, name=f"ps_{qt}_{kt}", tag="ps")
                        # P = -qk
                        for c in range(DC):
                            nc.tensor.matmul(
                                ps[:],
                                kt_tiles[c][:, ks],
                                q_tiles[(qt, c)][:],
                                start=(c == 0),
                                stop=(c == DC - 1),
                            )
                        # numerator N = qk + s, read before the aug matmul
                        n_t = np_p.tile([128, 512], f32, name=f"n_{qt}_{kt}", tag="n")
                        nc.scalar.activation(
                            n_t[:], ps[:], Act.Copy, bias=SMOOTH, scale=-1.0)
                        pending.append((ps, n_t, qt, kt))
                        n_tiles += 1
                        last = n_tiles == QT_TILES * KT_TILES
                        if flushed is not None and (len(pending) % GRP == 1 or last):
                            finish_group(flushed)
                            flushed = None
                        if len(pending) == GRP:
                            if last:
                                finish_group(pending)
                            else:
                                flushed = pending
                            pending = []

    nc.compile()
    return nc


_NC_CACHE = None


def _get_nc():
    global _NC_CACHE
    if _NC_CACHE is None:
        _NC_CACHE = build_nc()
    return _NC_CACHE


# ---------------------------------------------------------------- entrypoint

def kernel(q, k, v, _trace=False):
    q = np.asarray(q, dtype=np.float32)
    k = np.asarray(k, dtype=np.float32)
    v = np.asarray(v, dtype=np.float32)

    in_maps = _host_prepare(q, k, v)
    nc = _get_nc()

    from concourse.bass_utils import run_bass_kernel_spmd
    res = run_bass_kernel_spmd(
        nc, in_maps, core_ids=list(range(N_CORES)), trace=_trace)
    outs = [r['out'] for r in res.results]
    result = _host_finish(outs)
    if _trace:
        kernel.last_results = res
    return result


if __name__ == '__main__':
    rng = np.random.default_rng(0)
    q = rng.standard_normal((B, T, C, H, W), dtype=np.float32)
    k = rng.standard_normal((B, T, C, H, W), dtype=np.float32)
    v = rng.standard_normal((B, T, C, H, W), dtype=np.float32)
    o = kernel(q, k, v)
    print("out", o.shape, o.dtype, float(np.abs(o).mean()))


# revision 32
# speedup vs baseline: 1.0900x; 1.0900x over previous
"""Trainium2 Bass kernel for RelPatchAttention2D (THW).

Problem: q,k,v (4,16,16,128,128) f32. Patchify into 4096 patches/batch of
dim 1024. sim[q,k] = (qk+s)/(qq+kk-qk+s); tqk[k] = mean_q sim; out = tqk * v.

Sharding (no collectives): 8 cores = 4 batches x 2 key-halves. Each core:
full queries (4096) x its 2048 keys. Host prepares transposed bf16 patch
matrices (with two augmentation rows), gathers/unpatchifies outputs.

Per-core kernel (layout: keys on partitions, queries on free dim),
processing kt tiles in groups of 4:
  per (qt,kt) tile [128 keys x 512 queries]:
    PE:  8 bf16 matmuls (d-chunks; stationary -K^T, moving Q^T)
         accumulate P = -qk in PSUM
    ACT: N = -P + s   (PSUM->SBUF numerator read, overlapped)
  per group of 4 kt tiles (issued one tile into the next group):
    PE:  4 aug matmuls (K=2 rows: qq_q*1 + 1*kk_k) onto the 4 banks,
         row-tiled to 32-row groups via tile_position=(32i,0) so all four
         stream CONCURRENTLY (~1 matmul slot for 4 tiles) -> D = qq+kk-qk
    DVE: r = reciprocal_approx_fast(D)
         acc[:,qt] = sum_q N*r   (scalar_tensor_tensor with accum)
  tqk = rowsum(acc)/4096; out = (v*tqk)*(1/4096)  (DVE tensor_scalar)

Numerics: N comes from the PSUM qk itself, so qq/kk quantization (bf16)
only perturbs the denominator - a benign RELATIVE error on sim. The N*r
form keeps the reciprocal's error relative to sim as well (no catastrophic
cancellation in sum(A/D)-4096).
"""
import os
import sys

import numpy as np

sys.path.insert(0, '/opt/trn_rl_repo')

SMOOTH = 1e-05
B, T, C, H, W = 4, 16, 16, 128, 128
SH = SW = 16
PH = PW = 8
NPATCH = T * SH * SW        # 4096 patches per batch (queries)
DPATCH = C * PH * PW        # 1024
KEYS_PER_CORE = NPATCH // 2  # 2048
N_CORES = 8

QT_TILES = NPATCH // 512     # 8
KT_TILES = KEYS_PER_CORE // 128  # 16
DC = DPATCH // 128           # 8 contraction chunks
GRP = 4                      # kt tiles per aug group (row-tiled aug packing)


# ----------------------------------------------------------------- host side

def _patchify_mat(x):
    # (B,T,C,H,W) -> (B, 4096, 1024), patch index = ((t*16+sh)*16+sw)
    xp = x.reshape(B, T, C, SH, PH, SW, PW).transpose(0, 1, 3, 5, 2, 4, 6)
    return np.ascontiguousarray(xp).reshape(B, NPATCH, DPATCH)


def _unpatchify_mat(p):
    # (B, 4096, 1024) -> (B,T,C,H,W)
    x = p.reshape(B, T, SH, SW, C, PH, PW).transpose(0, 1, 4, 2, 5, 3, 6)
    return np.ascontiguousarray(x).reshape(B, T, C, H, W)


def _host_prepare(q, k, v):
    import ml_dtypes
    QP = _patchify_mat(q)
    KP = _patchify_mat(k)
    VP = _patchify_mat(v)
    qq = np.square(QP, dtype=np.float64).sum(-1).astype(np.float32)
    kk = np.square(KP, dtype=np.float64).sum(-1).astype(np.float32)

    in_maps = []
    for b in range(B):
        qta = np.concatenate(
            [QP[b].T,
             qq[b][None, :],
             np.ones((1, NPATCH), np.float32)], axis=0)
        qta = np.ascontiguousarray(qta).astype(ml_dtypes.bfloat16)
        for half in range(2):
            sl = slice(half * KEYS_PER_CORE, (half + 1) * KEYS_PER_CORE)
            kta = np.concatenate(
                [-KP[b, sl].T,
                 np.ones((1, KEYS_PER_CORE), np.float32),
                 kk[b, sl][None, :]], axis=0)
            kta = np.ascontiguousarray(kta).astype(ml_dtypes.bfloat16)
            in_maps.append({
                'qta': qta,
                'kta': kta,
                'vp': np.ascontiguousarray(VP[b, sl]),
            })
    return in_maps


def _host_finish(outs):
    full = np.empty((B, NPATCH, DPATCH), np.float32)
    for b in range(B):
        full[b, :KEYS_PER_CORE] = outs[2 * b]
        full[b, KEYS_PER_CORE:] = outs[2 * b + 1]
    return _unpatchify_mat(full)


# --------------------------------------------------------------- bass kernel

def build_nc():
    import concourse.bass as bass  # noqa: F401
    import concourse.mybir as mybir
    import concourse.tile as tile
    from concourse import bacc

    f32 = mybir.dt.float32
    bf16 = mybir.dt.bfloat16
    Alu = mybir.AluOpType
    Act = mybir.ActivationFunctionType

    nc = bacc.Bacc(
        "TRN2",
        target_bir_lowering=False,
        debug=False,
        enable_asserts=False,
        num_devices=N_CORES,
    )

    qta = nc.dram_tensor("qta", [DPATCH + 2, NPATCH], bf16, kind="ExternalInput").ap()
    kta = nc.dram_tensor("kta", [DPATCH + 2, KEYS_PER_CORE], bf16, kind="ExternalInput").ap()
    vp = nc.dram_tensor("vp", [KEYS_PER_CORE, DPATCH], f32, kind="ExternalInput").ap()
    out = nc.dram_tensor("out", [KEYS_PER_CORE, DPATCH], f32, kind="ExternalOutput").ap()

    with tile.TileContext(nc) as tc:
        with (
            tc.tile_pool(name="ktp", bufs=1) as ktp,
            tc.tile_pool(name="qp", bufs=2) as qp,
            tc.tile_pool(name="psp", bufs=8, space="PSUM") as psp,
            tc.tile_pool(name="np_", bufs=6) as np_p,
            tc.tile_pool(name="rp", bufs=5) as rp,
            tc.tile_pool(name="scrp", bufs=3) as scrp,
            tc.tile_pool(name="accp", bufs=1) as accp,
            tc.tile_pool(name="wp", bufs=2) as wp,
            tc.tile_pool(name="vvp", bufs=1) as vvp,
            tc.tile_pool(name="outp", bufs=3) as outp,
        ):
            # qt=0 moving tiles first so the first matmuls can start early
            q0_tiles = []
            for c in range(DC):
                t = qp.tile([128, 512], bf16, name=f"qtt{c}_0", tag=f"qtt{c}")
                nc.sync.dma_start(t[:], qta[c * 128:(c + 1) * 128, 0:512])
                q0_tiles.append(t)
            # aug rows replicated at partition offsets 0/32/64/96 for the
            # row-tiled aug matmuls
            q0_aug = qp.tile([98, 512], bf16, name="qaug_0", tag="qaug")
            for i in range(GRP):
                nc.sync.dma_start(
                    q0_aug[32 * i:32 * i + 2, :], qta[DPATCH:DPATCH + 2, 0:512])

            # resident -K^T chunks + aug rows; first 128 columns first (all
            # tile 0 needs), big loads via the idle GpSimd DMA queue
            kt_tiles = []
            for c in range(DC):
                t = ktp.tile([128, KEYS_PER_CORE], bf16, name=f"ktt{c}", tag=f"ktt{c}")
                nc.gpsimd.dma_start(t[:, 0:128], kta[c * 128:(c + 1) * 128, 0:128])
                kt_tiles.append(t)
            kt_aug = ktp.tile([98, KEYS_PER_CORE], bf16, name="ktaug", tag="ktaug")
            for i in range(GRP):
                nc.gpsimd.dma_start(
                    kt_aug[32 * i:32 * i + 2, :], kta[DPATCH:DPATCH + 2, :])
            for c in range(DC):
                nc.gpsimd.dma_start(
                    kt_tiles[c][:, 128:], kta[c * 128:(c + 1) * 128, 128:])

            # per-kt accumulators: one column per qt, reduced at the end
            acc_tiles = []
            for kt in range(KT_TILES):
                t = accp.tile([128, QT_TILES], f32, name=f"acc{kt}", tag=f"acc{kt}")
                acc_tiles.append(t)

            # value tiles: resident, loaded mid-kernel off the startup path
            v_tiles = [
                vvp.tile([128, DPATCH], f32, name=f"v_{kt}", tag=f"v{kt}")
                for kt in range(KT_TILES)
            ]

            q_augs = {0: q0_aug}

            def finish_kt(kt):
                red_t = wp.tile([128, 1], f32, name=f"red_{kt}", tag="red")
                nc.vector.tensor_reduce(
                    red_t[:], acc_tiles[kt][:],
                    op=Alu.add, axis=mybir.AxisListType.X)
                w_t = wp.tile([128, 1], f32, name=f"w_{kt}", tag="w")
                nc.scalar.activation(
                    w_t[:], red_t[:], Act.Copy, scale=1.0 / NPATCH)
                o_t = outp.tile([128, DPATCH], f32, name=f"o_{kt}", tag="o")
                # ACT is idle by the tail; keep the wide scale off the DVE
                nc.scalar.activation(o_t[:], v_tiles[kt][:], Act.Copy, scale=w_t[:])
                nc.sync.dma_start(out[kt * 128:(kt + 1) * 128, :], o_t[:])

            def finish_group(grp):
                """aug matmuls (row-tiled, concurrent) + recip + STT accum
                for a pending group of tiles."""
                qt = grp[0][2]
                # 4 K=2 aug matmuls on disjoint 32-row groups (tile_position
                # packing where the scheduler lets them land adjacently)
                for i, (ps, n_t, _qt, kt) in enumerate(grp):
                    ks = slice(kt * 128, (kt + 1) * 128)
                    nc.tensor.matmul(
                        ps[:],
                        kt_aug[32 * i:32 * i + 2, ks],
                        q_augs[qt][32 * i:32 * i + 2, :],
                        start=False, stop=True,
                        skip_group_check=True,
                        tile_position=(32 * i, 0),
                    )
                for (ps, n_t, _qt, kt) in grp:
                    r_t = rp.tile([128, 512], f32, name=f"r_{qt}_{kt}", tag="r")
                    nc.vector.reciprocal_approx_fast(r_t[:], ps[:])
                    scr = scrp.tile([128, 512], f32, name=f"scr_{qt}_{kt}", tag="scr")
                    nc.vector.scalar_tensor_tensor(
                        scr[:], n_t[:], 1.0, r_t[:],
                        op0=Alu.bypass, op1=Alu.mult,
                        accum_out=acc_tiles[kt][:, qt:qt + 1],
                    )
                    if qt == QT_TILES - 1:
                        finish_kt(kt)

            pending = []   # tiles awaiting aug: list of (ps, n_t, qt, kt)
            flushed = None
            for qt in range(QT_TILES):
                qs = slice(qt * 512, (qt + 1) * 512)
                if qt == 0:
                    q_tiles = q0_tiles
                else:
                    q_tiles = []
                    for c in range(DC):
                        t = qp.tile([128, 512], bf16, name=f"qtt{c}_{qt}", tag=f"qtt{c}")
                        nc.sync.dma_start(t[:], qta[c * 128:(c + 1) * 128, qs])
                        q_tiles.append(t)
                    q_aug = qp.tile([98, 512], bf16, name=f"qaug_{qt}", tag="qaug")
                    for i in range(GRP):
                        nc.sync.dma_start(
                            q_aug[32 * i:32 * i + 2, :], qta[DPATCH:DPATCH + 2, qs])
                    q_augs[qt] = q_aug
                if qt == 2:
                    for kt in range(KT_TILES):
                        nc.gpsimd.dma_start(
                            v_tiles[kt][:], vp[kt * 128:(kt + 1) * 128, :])

                for kt in range(KT_TILES):
                    ks = slice(kt * 128, (kt + 1) * 128)
                    ps = psp.tile([128, 512], f32, name=f"ps_{qt}_{kt}", tag="ps")
                    # P = -qk
                    for c in range(DC):
                        nc.tensor.matmul(
                            ps[:],
                            kt_tiles[c][:, ks],
                            q_tiles[c][:],
                            start=(c == 0),
                            stop=(c == DC - 1),
                        )
                    # numerator N = qk + s, read before the aug matmul
                    n_t = np_p.tile([128, 512], f32, name=f"n_{qt}_{kt}", tag="n")
                    nc.scalar.activation(
                        n_t[:], ps[:], Act.Copy, bias=SMOOTH, scale=-1.0)
                    pending.append((ps, n_t, qt, kt))
                    # flush the previous full group one tile into this group
                    if flushed is not None and len(pending) % GRP == 1:
                        finish_group(flushed)
                        flushed = None
                    if len(pending) == GRP:
                        if qt == QT_TILES - 1:
                            finish_group(pending)   # no delay on the last pass
                        else:
                            flushed = pending
                        pending = []
            if flushed is not None:
                finish_group(flushed)

    nc.compile()
    return nc


_NC_CACHE = None


def _get_nc():
    global _NC_CACHE
    if _NC_CACHE is None:
        _NC_CACHE = build_nc()
    return _NC_CACHE


# ---------------------------------------------------------------- entrypoint

def kernel(q, k, v, _trace=False):
    q = np.asarray(q, dtype=np.float32)
    k = np.asarray(k, dtype=np.float32)
    v = np.asarray(v, dtype=np.float32)

    in_maps = _host_prepare(q, k, v)
    nc = _get_nc()

    from concourse.bass_utils import run_bass_kernel_spmd
    res = None
    for attempt in range(3):
        try:
            res = run_bass_kernel_spmd(
                nc, in_maps, core_ids=list(range(N_CORES)), trace=_trace)
            break
        except Exception:
            # transient NRT_EXEC_UNIT_UNRECOVERABLE etc. — retry on a
            # recovered device
            if attempt == 2:
                raise
            import time
            time.sleep(2.0)
    outs = [r['out'] for r in res.results]
    result = _host_finish(outs)
    if _trace:
        kernel.last_results = res
    return result


if __name__ == '__main__':
    rng = np.random.default_rng(0)
    q = rng.standard_normal((B, T, C, H, W), dtype=np.float32)
    k = rng.standard_normal((B, T, C, H, W), dtype=np.float32)
    v = rng.standard_normal((B, T, C, H, W), dtype=np.float32)
    o = kernel(q, k, v)
    print("out", o.shape, o.dtype, float(np.abs(o).mean()))
